# revision 7
# baseline (speedup 1.0000x reference)
"""GQA attention (bs=2, seq=2048, dim=2048, 16 q-heads / 8 kv-heads, hd=128)
on 8 Trainium2 NeuronCores.

Sharding: 2-way data parallel (batch) x 4-way tensor parallel (heads, kv
groups intact).  Core c handles batch c//4 and q-heads [4*(c%4), 4*(c%4)+4)
(kv-heads [2*(c%4), 2*(c%4)+2)).  Each core computes a partial output
projection (row-split wo); the all-reduce over the 4 TP ranks is done on the
host while gathering.

Device kernel (per core, all matmuls in fp32r = full PE rate):
  - host supplies X^T (so `dim` lands on partitions for every projection)
    and rotate-half permuted wq/wk, so RoPE is 3 contiguous-partition DVE
    ops per tile.
  - scores are computed transposed (P^T[k, q]) which makes PV and the
    output projection transpose-free; softmax row-sums come from a
    ones-column matmul, normalization via reciprocal + broadcast matmul.
  - causal masking: k-chunks with k > q_max are skipped entirely; the 4
    diagonal chunk shapes use host-precomputed 0/1 masks multiplied into
    exp(scores).
"""

from contextlib import ExitStack

import numpy as np

import concourse.bass as bass
import concourse.tile as tile
from concourse import bacc, mybir
from concourse.bass_utils import run_bass_kernel_spmd

F32 = mybir.dt.float32
F32R = mybir.dt.float32r

BS = 2
SEQ = 2048
DIM = 2048
N_HEADS = 16
N_KV_HEADS = 8
HD = 128
HALF = HD // 2

NCORES = 8
TP = 4                     # tensor-parallel ranks per batch
NH = N_HEADS // TP         # q heads per core = 4
NKV = N_KV_HEADS // TP     # kv heads per core = 2
QB = 512                   # q block (free dim of score matmuls)
KC = 128                   # k chunk (partition dim of P^T tiles)
DC = 128                   # contraction chunk (partitions)
NDC = DIM // DC            # 16
NB = SEQ // QB             # 4 seq blocks
SCALE = 1.0 / np.sqrt(HD)


def _build_nc():
    nc = bacc.Bacc("TRN2", target_bir_lowering=False, debug=False,
                   num_devices=NCORES)
    xT_d = nc.declare_dram_parameter("xT", [DIM, SEQ], F32R, isOutput=False)
    wq_d = nc.declare_dram_parameter("wq", [DIM, NH * HD], F32R, isOutput=False)
    wk_d = nc.declare_dram_parameter("wk", [DIM, NKV * HD], F32R, isOutput=False)
    wv_d = nc.declare_dram_parameter("wv", [DIM, NKV * HD], F32R, isOutput=False)
    wo_d = nc.declare_dram_parameter("wo", [NH * HD, DIM], F32R, isOutput=False)
    cos_d = nc.declare_dram_parameter("cos2", [HD, SEQ], F32, isOutput=False)
    sin_d = nc.declare_dram_parameter("sins", [HD, SEQ], F32, isOutput=False)
    msk_d = nc.declare_dram_parameter("masks", [4, KC, QB], F32, isOutput=False)
    on128_d = nc.declare_dram_parameter("ones128", [128, 1], F32R, isOutput=False)
    on1_d = nc.declare_dram_parameter("ones1", [1, 128], F32R, isOutput=False)
    out_d = nc.declare_dram_parameter("out", [SEQ, DIM], F32, isOutput=True)

    with tile.TileContext(nc) as tc, ExitStack() as ctx:
        wpool = ctx.enter_context(tc.tile_pool(name="weights", bufs=1))
        kvpool = ctx.enter_context(tc.tile_pool(name="kv", bufs=1))
        xpool = ctx.enter_context(tc.tile_pool(name="xt", bufs=17))
        wopool = ctx.enter_context(tc.tile_pool(name="wo", bufs=5))
        qpool = ctx.enter_context(tc.tile_pool(name="qT", bufs=4))
        ppool = ctx.enter_context(tc.tile_pool(name="pT", bufs=2))
        opool = ctx.enter_context(tc.tile_pool(name="oT", bufs=4))
        cspool = ctx.enter_context(tc.tile_pool(name="cs", bufs=2))
        npool = ctx.enter_context(tc.tile_pool(name="norm", bufs=1))
        tpool = ctx.enter_context(tc.tile_pool(name="tmp", bufs=2))
        obpool = ctx.enter_context(tc.tile_pool(name="outb", bufs=2))
        ps_acc = ctx.enter_context(tc.tile_pool(name="ps_acc", bufs=4,
                                                space="PSUM"))
        ps_sc = ctx.enter_context(tc.tile_pool(name="ps_sc", bufs=2,
                                               space="PSUM"))
        ps_att = ctx.enter_context(tc.tile_pool(name="ps_att", bufs=2,
                                                space="PSUM"))

        # ---- persistent weights/constants in SBUF ----
        wq_sb = wpool.tile([128, NDC * NH * HD], F32R)   # [128, 8192]
        wk_sb = wpool.tile([128, NDC * NKV * HD], F32R)  # [128, 4096]
        wv_sb = wpool.tile([128, NDC * NKV * HD], F32R)  # [128, 4096]
        for d in range(NDC):
            nc.sync.dma_start(wq_sb[:, d * 512:(d + 1) * 512],
                              wq_d.ap()[d * 128:(d + 1) * 128, :])
            nc.sync.dma_start(wk_sb[:, d * 256:(d + 1) * 256],
                              wk_d.ap()[d * 128:(d + 1) * 128, :])
            nc.sync.dma_start(wv_sb[:, d * 256:(d + 1) * 256],
                              wv_d.ap()[d * 128:(d + 1) * 128, :])
        masks = []
        for i in range(4):
            m = wpool.tile([KC, QB], F32, tag=f"mask{i}", name=f"mask{i}")
            nc.sync.dma_start(m[:], msk_d.ap()[i])
            masks.append(m)
        ones128 = wpool.tile([128, 1], F32R, tag="ones128")
        nc.sync.dma_start(ones128[:], on128_d.ap()[:])
        ones1 = wpool.tile([1, 128], F32R, tag="ones1")
        nc.sync.dma_start(ones1[:], on1_d.ap()[:])

        # ---- persistent K^T / V for the whole sequence ----
        kT = [kvpool.tile([128, SEQ], F32R, tag=f"kT{g}", name=f"kT{g}")
              for g in range(NKV)]
        # v_sb columns: [kchunk c][kv head g] -> [:, c*256 + g*128 :+128]
        v_sb = kvpool.tile([128, (SEQ // KC) * NKV * HD], F32R, tag="v")
        assert v_sb.shape[1] == 4096

        def rope(dst, src_ps, cos_t, sin_t):
            """dst = src*cos2 + swap_halves(src)*sins  (dst f32r SBUF)."""
            tmp = tpool.tile([128, QB], F32, tag="ropetmp")
            nc.vector.tensor_mul(tmp[:], src_ps[:], cos_t[:])
            nc.vector.tensor_mul(dst[0:64, :], src_ps[64:128, :],
                                 sin_t[0:64, :])
            nc.vector.tensor_mul(dst[64:128, :], src_ps[0:64, :],
                                 sin_t[64:128, :])
            nc.vector.tensor_add(dst[:], dst[:], tmp[:])

        for j in range(NB):                       # seq blocks of 512
            c0 = j * QB
            cos_t = cspool.tile([128, QB], F32, tag="cos")
            sin_t = cspool.tile([128, QB], F32, tag="sin")
            nc.sync.dma_start(cos_t[:], cos_d.ap()[:, c0:c0 + QB])
            nc.sync.dma_start(sin_t[:], sin_d.ap()[:, c0:c0 + QB])

            # ---- Q projection + rope ----
            qT = []
            for h in range(NH):
                xts = []
                q_ps = ps_acc.tile([128, QB], F32, tag="acc")
                for d in range(NDC):
                    if h == 0:
                        xt = xpool.tile([128, QB], F32R, tag="xt")
                        nc.sync.dma_start(
                            xt[:], xT_d.ap()[d * 128:(d + 1) * 128,
                                             c0:c0 + QB])
                        xts.append(xt)
                    else:
                        xt = q_xts[d]
                    nc.tensor.matmul(
                        q_ps[:],
                        wq_sb[:, d * 512 + h * 128: d * 512 + (h + 1) * 128],
                        xt[:], start=(d == 0), stop=(d == NDC - 1))
                if h == 0:
                    q_xts = xts
                qt = qpool.tile([128, QB], F32R, tag="qT")
                rope(qt, q_ps, cos_t, sin_t)
                qT.append(qt)

            # ---- K projection + rope ----
            for g in range(NKV):
                k_ps = ps_acc.tile([128, QB], F32, tag="acc")
                for d in range(NDC):
                    nc.tensor.matmul(
                        k_ps[:],
                        wk_sb[:, d * 256 + g * 128: d * 256 + (g + 1) * 128],
                        q_xts[d][:], start=(d == 0), stop=(d == NDC - 1))
                rope(kT[g][:, c0:c0 + QB], k_ps, cos_t, sin_t)

            # ---- V projection (natural layout) ----
            for m in range(4):                    # 128-row seq chunks
                v_ps = ps_acc.tile([128, NKV * HD], F32, tag="acc")
                for d in range(NDC):
                    nc.tensor.matmul(
                        v_ps[:],
                        q_xts[d][:, m * 128:(m + 1) * 128],
                        wv_sb[:, d * 256:(d + 1) * 256],
                        start=(d == 0), stop=(d == NDC - 1))
                kc = 4 * j + m
                nc.scalar.copy(v_sb[:, kc * 256:(kc + 1) * 256], v_ps[:])

            # ---- attention for this q block ----
            nkc = 4 * (j + 1)
            oT = []
            for h in range(NH):
                g = h // 2
                o_ps = ps_att.tile([128, QB], F32, tag="att")
                z_ps = ps_att.tile([1, QB], F32, tag="att")
                for kc in range(nkc):
                    sc_ps = ps_sc.tile([128, QB], F32, tag="sc")
                    nc.tensor.matmul(sc_ps[:],
                                     kT[g][:, kc * 128:(kc + 1) * 128],
                                     qT[h][:], start=True, stop=True)
                    pt = ppool.tile([128, QB], F32R, tag="pT")
                    nc.scalar.activation(pt[:], sc_ps[:],
                                         mybir.ActivationFunctionType.Exp,
                                         scale=float(SCALE))
                    if kc >= 4 * j:
                        nc.vector.tensor_mul(pt[:], pt[:], masks[kc - 4 * j][:])
                    nc.tensor.matmul(o_ps[:],
                                     v_sb[:, kc * 256 + g * 128:
                                          kc * 256 + (g + 1) * 128],
                                     pt[:], start=(kc == 0),
                                     stop=(kc == nkc - 1))
                    nc.tensor.matmul(z_ps[:], ones128[:], pt[:],
                                     start=(kc == 0), stop=(kc == nkc - 1))
                # normalize: oT = o_ps / broadcast(z)
                z_sb = npool.tile([1, QB], F32R, tag="z")
                nc.scalar.copy(z_sb[:], z_ps[:])
                zb_ps = ps_sc.tile([128, QB], F32, tag="sc")
                nc.tensor.matmul(zb_ps[:], ones1[:], z_sb[:],
                                 start=True, stop=True)
                rz_sb = npool.tile([128, QB], F32, tag="rz")
                nc.vector.reciprocal(rz_sb[:], zb_ps[:])
                ot = opool.tile([128, QB], F32R, tag="oT")
                nc.vector.tensor_mul(ot[:], o_ps[:], rz_sb[:])
                oT.append(ot)

            # ---- partial output projection for this block's rows ----
            # wo streamed: tile (h, n) = wo rows [h*128,+128) cols [n*512,+512)
            for n in range(4):
                op_ps = [ps_acc.tile([128, 512], F32, tag="acc",
                                     name=f"op{j}_{n}_{m}") for m in range(4)]
                for h in range(NH):
                    wo_t = wopool.tile([128, 512], F32R, tag="wo",
                                       name=f"wo{j}_{n}_{h}")
                    nc.sync.dma_start(
                        wo_t[:], wo_d.ap()[h * 128:(h + 1) * 128,
                                           n * 512:(n + 1) * 512])
                    for m in range(4):
                        nc.tensor.matmul(
                            op_ps[m][:],
                            oT[h][:, m * 128:(m + 1) * 128],
                            wo_t[:],
                            start=(h == 0), stop=(h == NH - 1))
                for m in range(4):
                    ob = obpool.tile([128, 512], F32, tag="ob")
                    nc.scalar.copy(ob[:], op_ps[m][:])
                    nc.sync.dma_start(
                        out_d.ap()[c0 + m * 128: c0 + (m + 1) * 128,
                                   n * 512:(n + 1) * 512], ob[:])

    nc.compile()
    return nc


_NC_CACHE = None


def _get_nc():
    global _NC_CACHE
    if _NC_CACHE is None:
        _NC_CACHE = _build_nc()
    return _NC_CACHE


def _host_prep(inputs):
    """Build the 8 per-core input maps from the full problem inputs."""
    hs = np.asarray(inputs["hidden_state"], dtype=np.float32)
    cos = np.asarray(inputs["freq_cos"], dtype=np.float32)[0, :, 0, :]  # [S,64]
    sin = np.asarray(inputs["freq_sin"], dtype=np.float32)[0, :, 0, :]
    wq = np.asarray(inputs["wq"], dtype=np.float32)
    wk = np.asarray(inputs["wk"], dtype=np.float32)
    wv = np.asarray(inputs["wv"], dtype=np.float32)
    wo = np.asarray(inputs["wo"], dtype=np.float32)

    perm = np.concatenate([np.arange(0, HD, 2), np.arange(1, HD, 2)])  # [128]

    cos2 = np.empty((HD, SEQ), dtype=np.float32)
    sins = np.empty((HD, SEQ), dtype=np.float32)
    cos2[:HALF] = cos.T
    cos2[HALF:] = cos.T
    sins[:HALF] = -sin.T
    sins[HALF:] = sin.T

    qi = np.arange(QB)
    ki = np.arange(KC)
    masks = np.stack(
        [(ki[:, None] <= (qi[None, :] - 128 * i)).astype(np.float32)
         for i in range(4)])

    ones128 = np.ones((128, 1), dtype=np.float32)
    ones1 = np.ones((1, 128), dtype=np.float32)

    xTs = [np.ascontiguousarray(hs[b].T) for b in range(BS)]

    in_maps = []
    for c in range(NCORES):
        b, r = divmod(c, TP)
        qcols = np.concatenate(
            [(4 * r + h) * HD + perm for h in range(NH)])
        kcols = np.concatenate(
            [(NKV * r + g) * HD + perm for g in range(NKV)])
        vcols = np.concatenate(
            [(NKV * r + g) * HD + np.arange(HD) for g in range(NKV)])
        worows = np.concatenate(
            [(4 * r + h) * HD + np.arange(HD) for h in range(NH)])
        in_maps.append({
            "xT": xTs[b],
            "wq": np.ascontiguousarray(wq[:, qcols]),
            "wk": np.ascontiguousarray(wk[:, kcols]),
            "wv": np.ascontiguousarray(wv[:, vcols]),
            "wo": np.ascontiguousarray(wo[worows, :]),
            "cos2": cos2,
            "sins": sins,
            "masks": masks,
            "ones128": ones128,
            "ones1": ones1,
        })
    return in_maps


def _run(inputs, trace=False, **trace_kwargs):
    nc = _get_nc()
    in_maps = _host_prep(inputs)
    res = run_bass_kernel_spmd(nc, in_maps, list(range(NCORES)),
                               trace=trace, **trace_kwargs)
    out = np.zeros((BS, SEQ, DIM), dtype=np.float32)
    for c in range(NCORES):
        out[c // TP] += res.results[c]["out"]
    return out, res


def kernel(**inputs) -> np.ndarray:
    out, _ = _run(inputs, trace=False)
    return out
